# revision 6
# baseline (speedup 1.0000x reference)
"""Group-equivariant depthwise conv (C4) on 8 Trainium2 NeuronCores.

out[b, r*C+c] = crosscorr(x[b, c], rot90(weight[c, 0], r)), r in 0..3
x: [16, 192, 128, 128] f32, weight: [192, 1, 3, 3] f32 -> out: [16, 768, 128, 128].

Sharding: data-parallel over batch (2 images per core); per core the (b, c)
rows flatten to 384 partition-rows = 3 chunks of 128.

Algorithm (C4 symmetry-pair decomposition, all bf16 on chip):
  For the rotation pairs (0,2) and (1,3) the cross-corr kernels are 180deg
  rotations of each other, so with pair-sum planes s_k = x_a + x_b and
  pair-difference planes d_k = x_a - x_b over the 4 centrosymmetric tap
  pairs:
    p  = (out0+out2)/2 = sum_k alpha_k  s_k + alpha_c  x_center   (5 matmuls)
    m  = (out0-out2)/2 = sum_k beta_k   d_k                       (4 matmuls)
    p' = (out1+out3)/2 = sum_k alpha'_k s_k + alpha_c  x_center   (5 matmuls)
    m' = (out1-out3)/2 = sum_k beta'_k  d_k                       (4 matmuls)
    out0 = p + m, out2 = p - m, out1 = p' + m', out3 = p' - m'
  i.e. 18 diagonal-stationary matmuls produce all 4 rotations (vs 36 direct).
  The s/d planes are 8 DVE bf16 adds; combines are 4 DVE bf16 adds; the
  Scalar engine drains PSUM chain-pairs to SBUF bf16.

Input x is zero-padded (H+2, W+2) and converted to bf16 on the host, so
there are no on-chip memsets and loads/stores are large contiguous DMAs.
Output is stored as bf16 and upcast to f32 on the host.
"""

import numpy as np
from contextlib import ExitStack

from concourse import bacc, mybir, tile
from concourse.bass_utils import run_bass_kernel_spmd

B, C, H, W = 16, 192, 128, 128
NCORES = 8
BS = B // NCORES            # batches per core
ROWS = BS * C               # 384 (b,c) rows per core
NCHUNK = ROWS // 128        # 3
HT = 16                     # output rows per h-tile
NHT = H // HT               # 8
SUB = 4                     # output rows per PSUM chain-pair subtile
NSUB = HT // SUB            # 4
HP, WP = H + 2, W + 2       # host-padded image dims

F32 = mybir.dt.float32
BF16 = mybir.dt.bfloat16
NPBF16 = mybir.dt.np(BF16)

# centrosymmetric tap pairs (first, second) as (i, j) offsets in the 3x3 kernel
PAIRS = [((0, 0), (2, 2)), ((0, 1), (2, 1)), ((0, 2), (2, 0)), ((1, 2), (1, 0))]
CTR = (1, 1)

# w18 column layout: [alpha1..4, alpha_c, beta1..4, alpha'1..4, alpha'_c, beta'1..4]
NW18 = 18

# partition segments of each chunk: (p0, n, b_local, c0)
CHUNK_SEGS = []
for _ch in range(NCHUNK):
    segs = []
    g = _ch * 128
    while g < (_ch + 1) * 128:
        b_loc, c0 = g // C, g % C
        n = min((_ch + 1) * 128 - g, C - c0)
        segs.append((g - _ch * 128, n, b_loc, c0))
        g += n
    CHUNK_SEGS.append(segs)


def _build():
    nc = bacc.Bacc("TRN2", target_bir_lowering=False, debug=False, num_devices=NCORES)
    x_d = nc.dram_tensor("xp", [ROWS, HP, WP], BF16, kind="ExternalInput").ap()
    w_d = nc.dram_tensor("w18", [ROWS, NW18], F32, kind="ExternalInput").ap()
    o_d = nc.dram_tensor("out", [BS * 4 * C, H, W], BF16, kind="ExternalOutput").ap()

    with tile.TileContext(nc) as tc, ExitStack() as ctx:
        xpool = ctx.enter_context(tc.tile_pool(name="xraw", bufs=2))
        spool = ctx.enter_context(tc.tile_pool(name="sd", bufs=2))
        pmpool = ctx.enter_context(tc.tile_pool(name="pmsb", bufs=2))
        opool = ctx.enter_context(tc.tile_pool(name="osb", bufs=6))
        wpool = ctx.enter_context(tc.tile_pool(name="wsb", bufs=2))
        dpool = ctx.enter_context(tc.tile_pool(name="diag", bufs=1))
        pspool = ctx.enter_context(tc.tile_pool(name="ps", bufs=4, space="PSUM"))

        for ch in range(NCHUNK):
            g0 = ch * 128
            # per-chunk PM coefficients: [128, 18] f32 -> diag stationaries bf16
            w_sb = wpool.tile([128, NW18], F32, tag="wsb")
            nc.sync.dma_start(w_sb[:], w_d[g0 : g0 + 128, :])
            diag_f = dpool.tile([128, NW18, 128], F32, tag="df")
            nc.gpsimd.affine_select(
                out=diag_f[:],
                in_=w_sb[:].broadcast_to([128, NW18, 128]),
                compare_op=mybir.AluOpType.is_equal,
                fill=0.0,
                base=0,
                pattern=[[0, NW18], [-1, 128]],
                channel_multiplier=1,
            )
            diag = dpool.tile([128, NW18, 128], BF16, tag="db")
            nc.vector.tensor_copy(diag[:], diag_f[:])

            for ht in range(NHT):
                h0 = ht * HT
                # padded x tile: rows h0..h0+HT+1 of the padded image
                xt = xpool.tile([128, HT + 2, WP], BF16, tag="xraw")
                nc.sync.dma_start(xt[:], x_d[g0 : g0 + 128, h0 : h0 + HT + 2, :])

                # s/d planes over the 4 centrosymmetric pairs, output extent
                sd = spool.tile([128, 8, HT, W], BF16, tag="sd")
                for k, ((ai, aj), (bi, bj)) in enumerate(PAIRS):
                    xa = xt[:, ai : ai + HT, aj : aj + W]
                    xb = xt[:, bi : bi + HT, bj : bj + W]
                    nc.vector.tensor_tensor(out=sd[:, k], in0=xa, in1=xb,
                                            op=mybir.AluOpType.add)
                    nc.vector.tensor_tensor(out=sd[:, 4 + k], in0=xa, in1=xb,
                                            op=mybir.AluOpType.subtract)

                # pm[:, 0] = p-chain, pm[:, 1] = m-chain (bf16 staging)
                pmA = pmpool.tile([128, 2, HT, W], BF16, tag="pmA")
                pmB = pmpool.tile([128, 2, HT, W], BF16, tag="pmB")
                for pair_i, pm in ((0, pmA), (1, pmB)):
                    cbase = pair_i * 9  # alpha cols at cbase..cbase+4, beta at cbase+5..cbase+8
                    for s in range(NSUB):
                        r0 = SUB * s
                        ps = pspool.tile([128, 2, SUB, W], F32, tag="ps")
                        # p-chain: 4 pair-sum taps + center tap
                        for k in range(4):
                            nc.tensor.matmul(
                                ps[:, 0],
                                diag[:, cbase + k, :],
                                sd[:, k, r0 : r0 + SUB, :],
                                start=(k == 0),
                                stop=False,
                            )
                        nc.tensor.matmul(
                            ps[:, 0],
                            diag[:, cbase + 4, :],
                            xt[:, 1 + r0 : 1 + r0 + SUB, 1 : 1 + W],
                            start=False,
                            stop=True,
                        )
                        # m-chain: 4 pair-difference taps
                        for k in range(4):
                            nc.tensor.matmul(
                                ps[:, 1],
                                diag[:, cbase + 5 + k, :],
                                sd[:, 4 + k, r0 : r0 + SUB, :],
                                start=(k == 0),
                                stop=(k == 3),
                            )
                        # drain both chains to bf16 staging in one ACT op
                        nc.scalar.activation(
                            pm[:, :, r0 : r0 + SUB, :],
                            ps[:],
                            mybir.ActivationFunctionType.Copy,
                        )

                # combines + stores: out_r for r in (0, 2) from pmA, (1, 3) from pmB
                for pair_i, pm in ((0, pmA), (1, pmB)):
                    for sgn in (0, 1):
                        r = pair_i + 2 * sgn  # pmA -> rots 0/2, pmB -> rots 1/3
                        osb = opool.tile([128, HT, W], BF16, tag="osb")
                        nc.vector.tensor_tensor(
                            out=osb[:],
                            in0=pm[:, 0],
                            in1=pm[:, 1],
                            op=mybir.AluOpType.add if sgn == 0 else mybir.AluOpType.subtract,
                        )
                        for p0, n, b_loc, c0 in CHUNK_SEGS[ch]:
                            row0 = b_loc * 4 * C + r * C + c0
                            nc.sync.dma_start(
                                o_d[row0 : row0 + n, h0 : h0 + HT, :],
                                osb[p0 : p0 + n, :, :],
                            )

    nc.compile()
    return nc


_NC = None


def _get_nc():
    global _NC
    if _NC is None:
        _NC = _build()
    return _NC


def _make_w18(weight):
    """Per-channel PM coefficients: [C, 18] f32, tiled to [ROWS, 18]."""
    base = np.asarray(weight, dtype=np.float32)[:, 0]  # [C, 3, 3]
    K = [np.rot90(base, r, axes=(1, 2)) for r in range(4)]
    w18 = np.zeros((C, NW18), dtype=np.float32)
    for pair_i, (Ka, Kb) in enumerate(((K[0], K[2]), (K[1], K[3]))):
        cb = pair_i * 9
        for k, ((ai, aj), _) in enumerate(PAIRS):
            w18[:, cb + k] = 0.5 * (Ka[:, ai, aj] + Kb[:, ai, aj])
            w18[:, cb + 5 + k] = 0.5 * (Ka[:, ai, aj] - Kb[:, ai, aj])
        w18[:, cb + 4] = Ka[:, CTR[0], CTR[1]]  # == Kb center
    return np.tile(w18, (BS, 1))


def _make_in_maps(x, weight):
    x = np.asarray(x, dtype=np.float32)
    w18 = _make_w18(weight)
    xp = np.zeros((B * C, HP, WP), dtype=NPBF16)
    xp[:, 1 : 1 + H, 1 : 1 + W] = x.reshape(B * C, H, W).astype(NPBF16)
    xp = xp.reshape(NCORES, ROWS, HP, WP)
    return [{"xp": np.ascontiguousarray(xp[k]), "w18": w18} for k in range(NCORES)]


def kernel(x, weight):
    in_maps = _make_in_maps(x, weight)
    nc = _get_nc()
    res = run_bass_kernel_spmd(nc, in_maps, list(range(NCORES))).results
    out = np.empty((B, 4 * C, H, W), dtype=np.float32)
    for k in range(NCORES):
        out[BS * k : BS * (k + 1)] = (
            res[k]["out"].astype(np.float32).reshape(BS, 4 * C, H, W)
        )
    return out
